# revision 2
# baseline (speedup 1.0000x reference)
"""Causal self-attention (GQA + RoPE) Trainium2 kernel, v2.

Sharding: 8 cores = 4 batches x 2 query-shards (stride-2 interleave).
Core (b, j) computes batch b, query rows {j, j+2, ...}. K/V computed over
the full 2048-row prefix on both cores of a batch pair.

v2 changes vs v1:
  - x is transposed on the HOST (xT/xqT inputs, d-major) -> projection
    lhsT tiles load with plain large DMAs; no input DMA-transposes.
  - Roped q/k are transposed on-chip with PE-mode transpose (identity
    matmul) + ScalarE copy-back, replacing 112 serialized DMA-transposes.
  - Attention processes 512-wide query halves (qh) with k-block PAIRS:
    S^T for a pair lands in one [128, 2, 512] f32 PSUM tile and is
    exponentiated by ONE ScalarE call (halves ACT instruction overhead).
  - Softmax denominator broadcast via gpsimd.partition_broadcast
    (GpSimd is otherwise idle) instead of a DRAM roundtrip.
  - Output projection for query-half 0 is emitted between the two
    attention halves so it overlaps with attention of half 1.

Per-core pipeline:
  1. DMA xT [128,8,T], xqT [128,8,TQ] (host-transposed, bf16, d-padded
     to 1024 rows with zeros).
  2. K/V projection per k-tile (PE, lhsT = xT tile), RoPE on K (DVE,
     interleaved-pair APs on PSUM), V -> SBUF with ones column [V|1],
     K -> PE-transpose -> kT_sb (d on partitions, 2 kv-heads/group).
  3. Q projection per q-tile from xqT, RoPE, PE-transpose -> qT_sb.
  4. Attention per (q-half, head-slot, kb-pair): S^T = kT.T @ qT into
     [128,2,512] PSUM, exp on ACT (scale=1/8 folded; scores bounded so
     no max subtraction), 0/1 mask multiply on diagonal pairs, PV with
     lhsT=[V|1] accumulating O^T rows 0..63 + denominator in row 64.
  5. Normalize: reciprocal of row 64 (DVE) -> partition_broadcast
     (GpSimd) -> multiply (DVE) -> oT_sb bf16.
  6. Output projection per q-tile (PE, lhsT = oT_sb), DVE copy, DMA out.

Head-slot permutation (as v1): q-head h -> slot s with partition offset
64*(s%2) matching its kv head's kT offset; g4 duplicated for slot 13;
slot 15 is a zero dummy.
"""

import sys

if "/opt/trn_rl_repo" not in sys.path:
    sys.path.insert(0, "/opt/trn_rl_repo")

import numpy as np
import ml_dtypes

import concourse.bass as bass
import concourse.tile as tile
from concourse import bacc, mybir
from concourse.bass_utils import run_bass_kernel_spmd

BF16 = ml_dtypes.bfloat16

B, T, DIM = 4, 2048, 960
N_HEADS, N_KV_HEADS, HEAD_DIM = 15, 5, 64
DPAD = 1024          # padded model dim (zeros in rows 960:1024 of xT / weights)
NSLOT = 16           # q-head slots (15 real + 1 dummy)
TQ = 1024            # local query rows per core
NQT = TQ // 128      # 8 q-tiles
NKT = T // 128       # 16 k-blocks
SCALE = 1.0 / 8.0    # 1/sqrt(HEAD_DIM)

# q-head for each slot; chosen so 64*(s%2) == 64*((h//3)%2) except s=13 (g4 dup)
SLOT_HEAD = [0, 3, 1, 4, 2, 5, 6, 9, 7, 10, 8, 11, 12, 13, 14, None]

_CACHE = {}


def _build_program(phases=("kv", "q", "att"), rep=1):
    if isinstance(phases, dict):
        reps = phases
    else:
        reps = {p: rep for p in phases}
    nc = bacc.Bacc("TRN2", target_bir_lowering=False, debug=False,
                   enable_asserts=False)
    f32 = mybir.dt.float32
    bf = mybir.dt.bfloat16

    xT_d = nc.dram_tensor("xT", [DPAD, T], bf, kind="ExternalInput").ap()
    xqT_d = nc.dram_tensor("xqT", [DPAD, TQ], bf, kind="ExternalInput").ap()
    wq_d = nc.dram_tensor("wq", [DPAD, NSLOT * HEAD_DIM], bf, kind="ExternalInput").ap()
    wkv_d = nc.dram_tensor("wkv", [DPAD, 640], bf, kind="ExternalInput").ap()
    wo_d = nc.dram_tensor("wo", [DPAD, DIM], bf, kind="ExternalInput").ap()
    cosq_d = nc.dram_tensor("cosq", [TQ, 32], f32, kind="ExternalInput").ap()
    sinq_d = nc.dram_tensor("sinq", [TQ, 32], f32, kind="ExternalInput").ap()
    cosk_d = nc.dram_tensor("cosk", [T, 32], f32, kind="ExternalInput").ap()
    sink_d = nc.dram_tensor("sink", [T, 32], f32, kind="ExternalInput").ap()
    mask_d = nc.dram_tensor("maskT", [2, 128, 128], bf, kind="ExternalInput").ap()
    ident_d = nc.dram_tensor("ident", [128, 128], bf, kind="ExternalInput").ap()
    # bf16 output halves the per-call HBM/host traffic; host casts to f32.
    out_d = nc.dram_tensor("out", [TQ, DIM], bf, kind="ExternalOutput").ap()

    def bc(ap, n, axis):
        """Insert a stride-0 broadcast dim of size n at free-dim position axis."""
        a = list(ap.ap)
        a.insert(axis, [0, n])
        return bass.AP(tensor=ap.tensor, offset=ap.offset, ap=a)

    with tile.TileContext(nc) as tc:
        with (
            tc.tile_pool(name="consts", bufs=1) as consts,
            tc.tile_pool(name="rope", bufs=3) as ropep,
            tc.tile_pool(name="tmp", bufs=4) as tmpp,
            tc.tile_pool(name="pt", bufs=3) as ptp,
            tc.tile_pool(name="lnorm", bufs=3) as lnp,
            tc.tile_pool(name="ost", bufs=2) as ostp,
            tc.tile_pool(name="ps", bufs=4, space="PSUM") as psp,
        ):
            # ---- persistent SBUF tensors ----
            xT_sb = consts.tile([128, 8, T], bf)
            xqT_sb = consts.tile([128, 8, TQ], bf)
            wq_sb = consts.tile([128, 8, NSLOT * HEAD_DIM], bf)
            wkv_sb = consts.tile([128, 8, 640], bf)
            wo_sb = consts.tile([128, 8, DIM], bf)
            cosq_sb = consts.tile([128, NQT, 32], f32)
            sinq_sb = consts.tile([128, NQT, 32], f32)
            cosk_sb = consts.tile([128, NKT, 32], f32)
            sink_sb = consts.tile([128, NKT, 32], f32)
            mask_sb = consts.tile([128, 2, 128], bf)
            ident_sb = consts.tile([128, 128], bf)
            qT_sb = consts.tile([128, 8, TQ], bf)
            kT_sb = consts.tile([128, 3, T], bf)
            v_sb = consts.tile([128, NKT, N_KV_HEADS, HEAD_DIM + 1], bf)
            oT_sb = consts.tile([128, 8, TQ], bf)

            nc.sync.dma_start(out=xT_sb, in_=xT_d.rearrange("(a b) c -> b a c", a=8))
            nc.sync.dma_start(out=xqT_sb, in_=xqT_d.rearrange("(a b) c -> b a c", a=8))
            nc.sync.dma_start(out=wq_sb, in_=wq_d.rearrange("(a b) c -> b a c", a=8))
            nc.sync.dma_start(out=wkv_sb, in_=wkv_d.rearrange("(a b) c -> b a c", a=8))
            nc.sync.dma_start(out=wo_sb, in_=wo_d.rearrange("(a b) c -> b a c", a=8))
            nc.sync.dma_start(out=cosq_sb, in_=cosq_d.rearrange("(a b) c -> b a c", a=NQT))
            nc.sync.dma_start(out=sinq_sb, in_=sinq_d.rearrange("(a b) c -> b a c", a=NQT))
            nc.sync.dma_start(out=cosk_sb, in_=cosk_d.rearrange("(a b) c -> b a c", a=NKT))
            nc.sync.dma_start(out=sink_sb, in_=sink_d.rearrange("(a b) c -> b a c", a=NKT))
            nc.sync.dma_start(out=mask_sb, in_=mask_d.rearrange("a b c -> b a c"))
            nc.sync.dma_start(out=ident_sb, in_=ident_d)
            nc.vector.memset(v_sb[:, :, :, HEAD_DIM:HEAD_DIM + 1], 1.0)
            nc.vector.memset(oT_sb[64:128, 7, :], 0.0)  # dummy slot 15 region

            def rope(u_ps, n_heads, cos_ap, sin_ap, dest):
                """RoPE on PSUM tile u_ps [128, n_heads*64] -> dest (SBUF bf16),
                interleaved-pair layout: pairs are (2i, 2i+1) along head_dim."""
                ue = bass.AP(tensor=u_ps.tensor, offset=u_ps.offset,
                             ap=[u_ps.ap[0], [HEAD_DIM, n_heads], [2, 32]])
                uo = bass.AP(tensor=u_ps.tensor, offset=u_ps.offset + 1,
                             ap=[u_ps.ap[0], [HEAD_DIM, n_heads], [2, 32]])
                cb = bc(cos_ap, n_heads, 1)
                sb_ = bc(sin_ap, n_heads, 1)
                t1 = tmpp.tile([128, n_heads, 32], f32, tag="t1")
                t2 = tmpp.tile([128, n_heads, 32], f32, tag="t2")
                de = bass.AP(tensor=dest.tensor, offset=dest.offset,
                             ap=[dest.ap[0], [HEAD_DIM, n_heads], [2, 32]])
                do = bass.AP(tensor=dest.tensor, offset=dest.offset + 1,
                             ap=[dest.ap[0], [HEAD_DIM, n_heads], [2, 32]])
                nc.vector.tensor_mul(t1, ue, cb)
                nc.vector.tensor_mul(t2, uo, sb_)
                nc.vector.tensor_sub(de, t1, t2)
                nc.vector.tensor_mul(t1, ue, sb_)
                nc.vector.tensor_mul(t2, uo, cb)
                nc.vector.tensor_add(do, t1, t2)

            # ---- K/V projection + K rope + PE transposes, per k row-tile ----
            for ti in [t_ for _ in range(reps.get("kv", 0)) for t_ in range(NKT)]:
                kv_ps = psp.tile([128, 640], f32, tag="big")
                for db in range(8):
                    nc.tensor.matmul(kv_ps[:, 0:512],
                                     xT_sb[:, db, ti * 128:(ti + 1) * 128],
                                     wkv_sb[:, db, 0:512],
                                     start=(db == 0), stop=(db == 7))
                    nc.tensor.matmul(kv_ps[:, 512:640],
                                     xT_sb[:, db, ti * 128:(ti + 1) * 128],
                                     wkv_sb[:, db, 512:640],
                                     start=(db == 0), stop=(db == 7))
                # rope K into k_rope slots 0..4 + dup of 4 into slot 5
                k_rope = ropep.tile([128, 6, HEAD_DIM], bf, tag="krope")
                rope(kv_ps, N_KV_HEADS, cosk_sb[:, ti, :], sink_sb[:, ti, :],
                     k_rope[:, 0:N_KV_HEADS, :])
                nc.vector.tensor_copy(k_rope[:, 5, :], k_rope[:, 4, :])
                # V -> SBUF with ones column
                nc.vector.tensor_copy(
                    v_sb[:, ti, :, 0:HEAD_DIM],
                    kv_ps[:, 320:640].rearrange("p (g d) -> p g d", g=N_KV_HEADS))
                # kT via PE transpose (head pairs)
                for tau in range(3):
                    tr = psp.tile([128, 128], bf, tag="big")
                    nc.tensor.transpose(tr, k_rope[:, 2 * tau:2 * tau + 2, :],
                                        ident_sb)
                    nc.scalar.copy(kT_sb[:, tau, ti * 128:(ti + 1) * 128], tr)

            # ---- Q projection + rope + PE transposes, per q-tile ----
            for qt in [t_ for _ in range(reps.get("q", 0)) for t_ in range(NQT)]:
                q_ps = psp.tile([128, NSLOT * HEAD_DIM], f32, tag="big")
                for db in range(8):
                    nc.tensor.matmul(q_ps[:, 0:512],
                                     xqT_sb[:, db, qt * 128:(qt + 1) * 128],
                                     wq_sb[:, db, 0:512],
                                     start=(db == 0), stop=(db == 7))
                    nc.tensor.matmul(q_ps[:, 512:1024],
                                     xqT_sb[:, db, qt * 128:(qt + 1) * 128],
                                     wq_sb[:, db, 512:1024],
                                     start=(db == 0), stop=(db == 7))
                q_rope = ropep.tile([128, NSLOT, HEAD_DIM], bf, tag="qrope")
                rope(q_ps, NSLOT, cosq_sb[:, qt, :], sinq_sb[:, qt, :], q_rope)
                for tau in range(8):
                    tr = psp.tile([128, 128], bf, tag="big")
                    nc.tensor.transpose(tr, q_rope[:, 2 * tau:2 * tau + 2, :],
                                        ident_sb)
                    nc.scalar.copy(qT_sb[:, tau, qt * 128:(qt + 1) * 128], tr)

            # ---- attention (per q-half, head-slot, kb-pair) + out proj ----
            for _ in range(reps.get("att", 0)):
                for qh in range(2):
                    for s in range(NSLOT - 1):
                        h = SLOT_HEAD[s]
                        g = h // 3
                        qoff = 64 * (s % 2)
                        if 64 * (g % 2) == qoff:
                            ktau, koff = g // 2, 64 * (g % 2)
                        else:
                            ktau, koff = 2, 64  # duplicated g4
                        n_pairs = 4 * (qh + 1)
                        oT_ps = psp.tile([128, 512], f32, tag="big")
                        for m in range(n_pairs):
                            q0l = max(0, 128 * m - 512 * qh)
                            sT = psp.tile([128, 2, 512], f32, tag="big")
                            pT = ptp.tile([128, 2, 512], bf, tag="pT")
                            for i in range(2):
                                kb = 2 * m + i
                                nc.tensor.matmul(
                                    sT[:, i, q0l:512],
                                    kT_sb[koff:koff + 64, ktau,
                                          kb * 128:(kb + 1) * 128],
                                    qT_sb[qoff:qoff + 64, s // 2,
                                          512 * qh + q0l:512 * (qh + 1)],
                                    start=True, stop=True)
                            nc.scalar.activation(pT[:, :, q0l:512],
                                                 sT[:, :, q0l:512],
                                                 mybir.ActivationFunctionType.Exp,
                                                 bias=0.0, scale=SCALE)
                            if m >= 4 * qh:  # diagonal pair: causal mask
                                for i in range(2):
                                    nc.vector.tensor_mul(
                                        pT[:, i, q0l:q0l + 128],
                                        pT[:, i, q0l:q0l + 128],
                                        mask_sb[:, i, :])
                            for i in range(2):
                                kb = 2 * m + i
                                nc.tensor.matmul(
                                    oT_ps[0:65, q0l:512],
                                    v_sb[:, kb, g, :],
                                    pT[:, i, q0l:512],
                                    start=(m == 0 and i == 0),
                                    stop=(m == n_pairs - 1 and i == 1))
                        # normalize: 1/denominator, broadcast, multiply
                        linv = lnp.tile([1, 512], f32, tag="linv")
                        nc.vector.reciprocal(linv, oT_ps[64:65, :])
                        lbc = lnp.tile([64, 512], f32, tag="lbc")
                        nc.gpsimd.partition_broadcast(lbc, linv[0:1, :])
                        nc.vector.tensor_mul(
                            oT_sb[qoff:qoff + 64, s // 2,
                                  512 * qh:512 * (qh + 1)],
                            oT_ps[0:64, :], lbc)
                    # output projection for this q-half (overlaps next half)
                    for qt in range(4 * qh, 4 * (qh + 1)):
                        o_ps = psp.tile([128, DIM], f32, tag="big")
                        for kt in range(8):
                            nc.tensor.matmul(
                                o_ps[:, 0:512],
                                oT_sb[:, kt, qt * 128:(qt + 1) * 128],
                                wo_sb[:, kt, 0:512],
                                start=(kt == 0), stop=(kt == 7))
                            nc.tensor.matmul(
                                o_ps[:, 512:960],
                                oT_sb[:, kt, qt * 128:(qt + 1) * 128],
                                wo_sb[:, kt, 512:960],
                                start=(kt == 0), stop=(kt == 7))
                        ost = ostp.tile([128, DIM], bf, tag="ost")
                        nc.vector.tensor_copy(ost, o_ps)
                        nc.sync.dma_start(out=out_d[qt * 128:(qt + 1) * 128, :],
                                          in_=ost)
            if not reps.get("att", 0):
                ost = ostp.tile([128, DIM], bf, tag="ost")
                nc.vector.memset(ost, 0.0)
                nc.sync.dma_start(out=out_d[0:128, :], in_=ost)

    nc.finalize()
    return nc


def _host_prep(x, freqs_cos, freqs_sin, wq, wk, wv, wo):
    """Build the shared + per-core input arrays (all numpy, host-side)."""
    xp = np.zeros((B, DPAD, T), dtype=BF16)
    for b in range(B):
        xp[b, :DIM, :] = x[b].T.astype(BF16)

    wqp = np.zeros((DPAD, NSLOT * HEAD_DIM), dtype=BF16)
    for s, h in enumerate(SLOT_HEAD):
        if h is None:
            continue
        wqp[:DIM, s * 64:(s + 1) * 64] = wq[:, h * 64:(h + 1) * 64].astype(BF16)

    wkvp = np.zeros((DPAD, 640), dtype=BF16)
    wkvp[:DIM, 0:320] = wk.astype(BF16)
    wkvp[:DIM, 320:640] = wv.astype(BF16)

    wop = np.zeros((DPAD, DIM), dtype=BF16)
    for s, h in enumerate(SLOT_HEAD):
        if h is None:
            continue
        r = 128 * (s // 2) + 64 * (s % 2)
        wop[r:r + 64, :] = wo[h * 64:(h + 1) * 64, :].astype(BF16)

    cosk = np.ascontiguousarray(freqs_cos, dtype=np.float32)
    sink = np.ascontiguousarray(freqs_sin, dtype=np.float32)
    ident = np.eye(128, dtype=BF16)

    shared = dict(wq=wqp, wkv=wkvp, wo=wop, cosk=cosk, sink=sink, ident=ident)

    in_maps = []
    for c in range(8):
        b, j = c // 2, c % 2
        m = dict(shared)
        m["xT"] = np.ascontiguousarray(xp[b])
        m["xqT"] = np.ascontiguousarray(xp[b][:, j::2])
        m["cosq"] = np.ascontiguousarray(cosk[j::2])
        m["sinq"] = np.ascontiguousarray(sink[j::2])
        kk = np.arange(128)[None, :, None]          # k index within block
        p = np.arange(128)[None, None, :]           # q row within tile
        mhalf = np.arange(2)[:, None, None] * 128
        mask = ((mhalf + kk) <= (2 * p + j)).astype(BF16)
        m["maskT"] = np.ascontiguousarray(mask)
        in_maps.append(m)
    return in_maps


def kernel(x, freqs_cos, freqs_sin, wq, wk, wv, wo):
    if "nc" not in _CACHE:
        _CACHE["nc"] = _build_program()
    nc = _CACHE["nc"]
    in_maps = _host_prep(np.asarray(x), np.asarray(freqs_cos),
                         np.asarray(freqs_sin), np.asarray(wq),
                         np.asarray(wk), np.asarray(wv), np.asarray(wo))
    res = run_bass_kernel_spmd(nc, in_maps, core_ids=list(range(8)))
    out = np.empty((B, T, DIM), dtype=np.float32)
    for c in range(8):
        b, j = c // 2, c % 2
        out[b, j::2, :] = np.asarray(res.results[c]["out"]).astype(np.float32)
    return out
